# revision 31
# baseline (speedup 1.0000x reference)
"""Trainium2 Bass kernel for DSAM-style strip-pooling attention recalibration.

Math (reference):
    S_h = mean(x, axis=W)                      # (B,C,H,1)
    S_v = mean(x, axis=H)                      # (B,C,1,W)
    F   = wh*S_h + wv*S_v                      # broadcast (B,C,H,W)
    Z   = relu(bn(w1 @ F))                     # (B,CR,H,W)
    A   = gelu(w2 @ Z)                         # (B,C,H,W)
    out = x + ls * (x * A) = x * (1 + ls*A)

Key restructuring: w1 is linear, so w1 @ (wh*S_h + wv*S_v) splits into
    Ph[b,o,h] = (wh/W * gs*w1) @ rowsum_w(x)   (per-h 16-vector)
    Pv[b,o,w] = (wv/H * gs*w1) @ colsum_h(x)   (per-w 16-vector)
with the BN affine folded into the weights (gs) and a bias gb. The full
F tensor is never materialized. Then per (h,w):
    t = relu(Ph[:,h] + Pv[:,w]);  A = gelu(w2 @ t);  out = x*(1+ls*A)

Sharding: H is split across the 8 cores (32 rows each). Row sums (for
Ph) are local; column sums (for Pv) are partial per core and combined
with four tiny (16 x 256) AllReduces, one per batch so each collective
overlaps the pooling of later batches. Pv partials are computed on the
TensorEngine by accumulating w1v^T @ x_bf16[b,:,h,:] over local h,
fusing strip pool + 1x1 conv into the matmul.

The first NCACHE x tiles stay resident in SBUF after the pooling pass so
the recalibration pass re-reads only the tail from HBM; streamed
recalibration tiles recycle the cache slots as they free up.
"""

import functools
import numpy as np

B, C, H, W = 4, 256, 256, 256
CR = 16
N_CORES = 8
H_SH = H // N_CORES          # 32 h-rows per core
HB = 8                       # h-rows per tile
NHB = H_SH // HB             # 4 tile-blocks per core
BN_EPS = 1e-5
NCH = C // 128               # 2 partition chunks of the channel dim
NT = B * NCH * NHB           # 32 x-tiles per core
NCACHE = 17                  # x tiles kept resident between passes


def _tile_index(b, ch, hb):
    return (b * NCH + ch) * NHB + hb


@functools.lru_cache(maxsize=1)
def _build():
    import concourse.bacc as bacc
    import concourse.mybir as mybir
    import concourse.tile as tile

    f32 = mybir.dt.float32
    bf16 = mybir.dt.bfloat16
    AF = mybir.ActivationFunctionType
    ALU = mybir.AluOpType

    nc = bacc.Bacc("TRN2", target_bir_lowering=False, debug=False,
                   num_devices=N_CORES)

    x_d = nc.dram_tensor("x", [B, C, H_SH, W], f32, kind="ExternalInput")
    w1v_d = nc.dram_tensor("w1v", [C, CR], bf16, kind="ExternalInput")
    w1h_d = nc.dram_tensor("w1h", [C, CR], f32, kind="ExternalInput")
    w2t_d = nc.dram_tensor("w2t", [CR, C], bf16, kind="ExternalInput")
    gb_d = nc.dram_tensor("gb", [CR, 1], f32, kind="ExternalInput")
    ls_d = nc.dram_tensor("ls", [C, 1], f32, kind="ExternalInput")
    y_d = nc.dram_tensor("y", [B, C, H_SH, W], f32, kind="ExternalOutput")

    with tile.TileContext(nc) as tc:
        with (
            tc.tile_pool(name="consts", bufs=1) as consts,
            tc.tile_pool(name="persist", bufs=1) as persist,
            tc.tile_pool(name="dram", bufs=1, space="DRAM") as dram,
            tc.tile_pool(name="xcache", bufs=1) as xcache,
            tc.tile_pool(name="xb", bufs=3) as xb_pool,
            tc.tile_pool(name="tb", bufs=3) as t_pool,
            tc.tile_pool(name="ab", bufs=2) as a_pool,
            tc.tile_pool(name="vb", bufs=2) as v_pool,
        ):
            w1v_sb = consts.tile([128, NCH * CR], bf16)
            w1h_sb = consts.tile([128, NCH * CR], f32)
            w2t_sb = consts.tile([CR, C], bf16)
            gb_sb = consts.tile([CR, 1], f32)
            ls_sb = consts.tile([128, NCH], f32)
            for ch in range(NCH):
                c0 = ch * 128
                nc.sync.dma_start(w1v_sb[:, ch * CR:(ch + 1) * CR],
                                  w1v_d[c0:c0 + 128, :])
                nc.sync.dma_start(w1h_sb[:, ch * CR:(ch + 1) * CR],
                                  w1h_d[c0:c0 + 128, :])
                nc.sync.dma_start(ls_sb[:, ch:ch + 1], ls_d[c0:c0 + 128, :])
            nc.sync.dma_start(w2t_sb[:], w2t_d[:, :])
            nc.sync.dma_start(gb_sb[:], gb_d[:, :])

            # persistent small tensors
            s_h_sb = persist.tile([128, NCH * B * H_SH], f32)   # row sums
            ph_sb = persist.tile([CR, B * H_SH], f32)           # Ph + gb
            pv_part_sb = persist.tile([CR, B * W], f32)         # local Pv
            pv_sb = persist.tile([CR, B * W], f32)              # reduced Pv

            pv_in_dr = [dram.tile([CR, W], f32, name=f"pv_in{b}",
                                  tag=f"pvi{b}") for b in range(B)]
            pv_out_dr = [dram.tile([CR, W], f32, name=f"pv_out{b}",
                                   tag=f"pvo{b}") for b in range(B)]

            # Warm up the collective firmware: the first AllReduce of a NEFF
            # pays ~35us of startup; eat it under the phase-A DMA ramp.
            warm_in = dram.tile([CR, 4], f32, name="warm_in", tag="wi")
            warm_out = dram.tile([CR, 4], f32, name="warm_out", tag="wo")
            nc.gpsimd.collective_compute(
                "AllReduce", ALU.add,
                replica_groups=[list(range(N_CORES))],
                ins=[warm_in[:].opt()], outs=[warm_out[:].opt()])

            x_tiles = {}   # tile index -> resident SBUF tile (first NCACHE)

            psA_cm = tc.tile_pool(name="psA", bufs=2, space="PSUM")
            psA = psA_cm.__enter__()
            psC_cm = tc.tile_pool(name="psC", bufs=2, space="PSUM")
            psC = psC_cm.__enter__()

            def emit_A(b):
                """Pooling pass for batch b, ending in its Pv AllReduce."""
                # [16, 512]: two w-copies (even/odd h rows), folded at the end
                psum_pv = psA.tile([CR, 2 * W], f32, name=f"psum_pv{b}",
                                   tag="pv")
                psum_ph = psA.tile([CR, H_SH], f32, name=f"psum_ph{b}",
                                   tag="ph")
                for ch in range(NCH):
                    c0 = ch * 128
                    for hb in range(NHB):
                        ti = _tile_index(b, ch, hb)
                        col = ch * B * H_SH + b * H_SH + hb * HB
                        if ti < NCACHE:
                            # resident fp32 tile + DVE bf16 cast
                            xt = xcache.tile([128, HB * W], f32,
                                             name=f"xc{ti}", tag=f"slot{ti}")
                            x_tiles[ti] = xt
                            nc.sync.dma_start(
                                xt[:],
                                x_d[b, c0:c0 + 128, hb * HB:(hb + 1) * HB, :])
                            nc.vector.tensor_reduce(
                                out=s_h_sb[:, col:col + HB],
                                in_=xt[:].rearrange("p (h w) -> p h w", w=W),
                                axis=mybir.AxisListType.X, op=ALU.add)
                            xbt = xb_pool.tile([128, HB * W], bf16,
                                               name="xb_t", tag="xb")
                            nc.scalar.copy(xbt[:], xt[:])
                        else:
                            # streamed: SWDGE casting DMA loads bf16 only
                            xbt = xb_pool.tile([128, HB * W], bf16,
                                               name="xb_t", tag="xb")
                            nc.gpsimd.dma_start(
                                xbt[:],
                                x_d[b, c0:c0 + 128, hb * HB:(hb + 1) * HB, :])
                            nc.vector.tensor_reduce(
                                out=s_h_sb[:, col:col + HB],
                                in_=xbt[:].rearrange("p (h w) -> p h w", w=W),
                                axis=mybir.AxisListType.X, op=ALU.add)
                        for k in range(HB // 2):
                            nc.tensor.matmul(
                                psum_pv[:, :],
                                w1v_sb[:, ch * CR:(ch + 1) * CR],
                                xbt[:, 2 * k * W:2 * (k + 1) * W],
                                start=(ch == 0 and hb == 0 and k == 0),
                                stop=(ch == NCH - 1 and hb == NHB - 1
                                      and k == HB // 2 - 1))
                for ch in range(NCH):
                    col = ch * B * H_SH + b * H_SH
                    nc.tensor.matmul(
                        psum_ph[:, :],
                        w1h_sb[:, ch * CR:(ch + 1) * CR],
                        s_h_sb[:, col:col + H_SH],
                        start=(ch == 0), stop=(ch == NCH - 1))
                nc.scalar.activation(ph_sb[:, b * H_SH:(b + 1) * H_SH],
                                     psum_ph[:, :], AF.Identity,
                                     bias=gb_sb[:, 0:1], scale=1.0)
                # per-batch AllReduce of the Pv partial (16 KiB) so the
                # collective overlaps the pooling of later batches
                nc.scalar.copy(pv_part_sb[:, b * W:(b + 1) * W],
                               psum_pv[:, 0:W])
                nc.vector.tensor_add(pv_part_sb[:, b * W:(b + 1) * W],
                                     pv_part_sb[:, b * W:(b + 1) * W],
                                     psum_pv[:, W:2 * W])
                # store + collective live on the gpsimd stream so the Sync
                # engine's load queue is never blocked behind their waits
                nc.gpsimd.dma_start(pv_in_dr[b][:],
                                    pv_part_sb[:, b * W:(b + 1) * W])
                nc.gpsimd.collective_compute(
                    "AllReduce", ALU.add,
                    replica_groups=[list(range(N_CORES))],
                    ins=[pv_in_dr[b][:].opt()],
                    outs=[pv_out_dr[b][:].opt()])

            def emit_C_prefetch(b):
                """Issue batch b's streamed phase-C reloads from ScalarE.
                Emitted after an earlier batch's compute so the recycled
                slots' final readers (y-stores on Sync) precede them."""
                for ch in range(NCH):
                    for hb in range(NHB):
                        ti = _tile_index(b, ch, hb)
                        if ti < NCACHE:
                            continue
                        c0 = ch * 128
                        xt = xcache.tile(
                            [128, HB * W], f32, name=f"xs{ti}",
                            tag=f"slot{(ti - NCACHE) % NCACHE}")
                        x_tiles[ti] = xt
                        nc.scalar.dma_start(
                            xt[:],
                            x_d[b, c0:c0 + 128, hb * HB:(hb + 1) * HB, :])

            def emit_C(b):
                """Recalibration pass for batch b."""
                # pv readback issued from ScalarE (also HWDGE-capable); by
                # emission time collective b has had a full batch to finish
                nc.scalar.dma_start(pv_sb[:, b * W:(b + 1) * W],
                                    pv_out_dr[b][:])
                HWH = 1024   # half-tile free size
                for hb in range(NHB):
                    tb = t_pool.tile([CR, HB * W], bf16, name="t_t",
                                     tag="tb")
                    for k in range(HB):
                        col = b * H_SH + hb * HB + k
                        nc.scalar.activation(
                            tb[:, k * W:(k + 1) * W],
                            pv_sb[:, b * W:(b + 1) * W],
                            AF.Relu, bias=ph_sb[:, col:col + 1], scale=1.0)
                    for ch in range(NCH):
                        c0 = ch * 128
                        ti = _tile_index(b, ch, hb)
                        xt = x_tiles[ti]   # resident or prefetched
                        vb = v_pool.tile([128, HB * W], f32,
                                         name="v_t", tag="vb")
                        ab = a_pool.tile([128, HB * W], bf16,
                                         name="a_t", tag="ab")
                        for half in range(2):
                            hof = half * HWH
                            ps = psC.tile([128, HWH], f32, name="ps_t",
                                          tag="ps")
                            for j in range(2):
                                nc.tensor.matmul(
                                    ps[:, j * 512:(j + 1) * 512],
                                    w2t_sb[:, c0:c0 + 128],
                                    tb[:, hof + j * 512:hof + (j + 1) * 512],
                                    start=True, stop=True)
                            nc.scalar.activation(ab[:, hof:hof + HWH], ps[:],
                                                 AF.Gelu)
                        # v = ls*A + 1 (per-partition ls), then out = x*v
                        nc.vector.tensor_scalar(
                            out=vb[:], in0=ab[:],
                            scalar1=ls_sb[:, ch:ch + 1], scalar2=1.0,
                            op0=ALU.mult, op1=ALU.add)
                        nc.vector.tensor_mul(xt[:], xt[:], vb[:])
                        nc.sync.dma_start(
                            y_d[b, c0:c0 + 128, hb * HB:(hb + 1) * HB, :],
                            xt[:])

            # All pooling first (the collective firmware needs ~70us to
            # come up, so phase C can't start early anyway), then the
            # recalibration batches, each prefetching the reloads of the
            # batch two steps ahead.
            for b in range(B):
                emit_A(b)
            for b in range(B):
                emit_C(b)
                if b + 2 < B:
                    emit_C_prefetch(b + 2)

            psC_cm.__exit__(None, None, None)
            psA_cm.__exit__(None, None, None)
    nc.compile()
    return nc


def _prepare(x, w1, w2, bn_gamma, bn_beta, bn_mean, bn_var, weight_h,
             weight_v, layer_scale):
    import ml_dtypes
    x = np.asarray(x, dtype=np.float32)
    w1 = np.asarray(w1, dtype=np.float32)
    w2 = np.asarray(w2, dtype=np.float32)
    inv_std = 1.0 / np.sqrt(np.asarray(bn_var, np.float32) + BN_EPS)
    gs = np.asarray(bn_gamma, np.float32) * inv_std
    gb = (np.asarray(bn_beta, np.float32)
          - np.asarray(bn_mean, np.float32) * gs)
    w1s = w1 * gs[:, None]                       # BN scale folded (CR, C)
    wh = float(np.asarray(weight_h).reshape(-1)[0])
    wv = float(np.asarray(weight_v).reshape(-1)[0])
    w1h_t = np.ascontiguousarray(w1s.T * (wh / W)).astype(np.float32)
    w1v_t = np.ascontiguousarray(w1s.T * (wv / H)).astype(ml_dtypes.bfloat16)
    w2t = np.ascontiguousarray(w2.T).astype(ml_dtypes.bfloat16)
    ls = np.ascontiguousarray(
        np.asarray(layer_scale, np.float32).reshape(C, 1))
    gb = np.ascontiguousarray(gb.reshape(CR, 1))
    in_maps = []
    for i in range(N_CORES):
        in_maps.append({
            "x": np.ascontiguousarray(x[:, :, i * H_SH:(i + 1) * H_SH, :]),
            "w1v": w1v_t, "w1h": w1h_t, "w2t": w2t, "gb": gb, "ls": ls,
        })
    return in_maps


def _run(in_maps, **kwargs):
    from concourse.bass_utils import run_bass_kernel_spmd
    nc = _build()
    return run_bass_kernel_spmd(nc, in_maps, core_ids=list(range(N_CORES)),
                                **kwargs)


def kernel(x, w1, w2, bn_gamma, bn_beta, bn_mean, bn_var, weight_h,
           weight_v, layer_scale):
    in_maps = _prepare(x, w1, w2, bn_gamma, bn_beta, bn_mean, bn_var,
                       weight_h, weight_v, layer_scale)
    res = _run(in_maps)
    y = np.empty((B, C, H, W), dtype=np.float32)
    for i in range(N_CORES):
        y[:, :, i * H_SH:(i + 1) * H_SH, :] = res.results[i]["y"]
    return y


# revision 37
# speedup vs baseline: 1.2749x; 1.2749x over previous
"""Trainium2 Bass kernel for DSAM-style strip-pooling attention recalibration.

Math (reference):
    S_h = mean(x, axis=W)                      # (B,C,H,1)
    S_v = mean(x, axis=H)                      # (B,C,1,W)
    F   = wh*S_h + wv*S_v                      # broadcast (B,C,H,W)
    Z   = relu(bn(w1 @ F))                     # (B,CR,H,W)
    A   = gelu(w2 @ Z)                         # (B,C,H,W)
    out = x + ls * (x * A) = x * (1 + ls*A)

Key restructuring: w1 is linear, so w1 @ (wh*S_h + wv*S_v) splits into
    Ph[b,o,h] = (wh/W * gs*w1) @ rowsum_w(x)   (per-h 16-vector)
    Pv[b,o,w] = (wv/H * gs*w1) @ colsum_h(x)   (per-w 16-vector)
with the BN affine folded into the weights (gs) and a bias gb. The full
F tensor is never materialized. Then per (h,w):
    t = relu(Ph[:,h] + Pv[:,w]);  A = gelu(w2 @ t);  out = x*(1+ls*A)

Sharding: H is split across the 8 cores (32 rows each). Row sums (for
Ph) are local; column sums (for Pv) are partial per core and combined
with four tiny (16 x 256) AllReduces, one per batch so each collective
overlaps the pooling of later batches. Pv partials are computed on the
TensorEngine by accumulating w1v^T @ x_bf16[b,:,h,:] over local h,
fusing strip pool + 1x1 conv into the matmul.

The first NCACHE x tiles stay resident in SBUF after the pooling pass so
the recalibration pass re-reads only the tail from HBM; streamed
recalibration tiles recycle the cache slots as they free up.
"""

import functools
import numpy as np

B, C, H, W = 4, 256, 256, 256
CR = 16
N_CORES = 8
H_SH = H // N_CORES          # 32 h-rows per core
HB = 8                       # h-rows per tile
NHB = H_SH // HB             # 4 tile-blocks per core
BN_EPS = 1e-5
NCH = C // 128               # 2 partition chunks of the channel dim
NT = B * NCH * NHB           # 32 x-tiles per core
NCACHE = 17                  # x tiles kept resident between passes


def _tile_index(b, ch, hb):
    return (b * NCH + ch) * NHB + hb


@functools.lru_cache(maxsize=1)
def _build():
    import concourse.bacc as bacc
    import concourse.mybir as mybir
    import concourse.tile as tile

    f32 = mybir.dt.float32
    bf16 = mybir.dt.bfloat16
    AF = mybir.ActivationFunctionType
    ALU = mybir.AluOpType

    from concourse.tile_rust import add_dep_helper

    nc = bacc.Bacc("TRN2", target_bir_lowering=False, debug=False,
                   num_devices=N_CORES)

    x_d = nc.dram_tensor("x", [B, C, H_SH, W], f32, kind="ExternalInput")
    w1v_d = nc.dram_tensor("w1v", [C, CR], bf16, kind="ExternalInput")
    w1h_d = nc.dram_tensor("w1h", [C, CR], f32, kind="ExternalInput")
    w2t_d = nc.dram_tensor("w2t", [CR, C], bf16, kind="ExternalInput")
    gb_d = nc.dram_tensor("gb", [CR, 1], f32, kind="ExternalInput")
    ls_d = nc.dram_tensor("ls", [C, 1], f32, kind="ExternalInput")
    y_d = nc.dram_tensor("y", [B, C, H_SH, W], f32, kind="ExternalOutput")

    with tile.TileContext(nc) as tc:
        with (
            tc.tile_pool(name="consts", bufs=1) as consts,
            tc.tile_pool(name="persist", bufs=1) as persist,
            tc.tile_pool(name="dram", bufs=1, space="DRAM") as dram,
            tc.tile_pool(name="xcache", bufs=1) as xcache,
            tc.tile_pool(name="xb", bufs=3) as xb_pool,
            tc.tile_pool(name="tb", bufs=3) as t_pool,
            tc.tile_pool(name="ab", bufs=2) as a_pool,
            tc.tile_pool(name="vb", bufs=2) as v_pool,
        ):
            w1v_sb = consts.tile([128, NCH * CR], bf16)
            w1h_sb = consts.tile([128, NCH * CR], f32)
            w2t_sb = consts.tile([CR, C], bf16)
            gb_sb = consts.tile([CR, 1], f32)
            ls_sb = consts.tile([128, NCH], f32)
            for ch in range(NCH):
                c0 = ch * 128
                nc.sync.dma_start(w1v_sb[:, ch * CR:(ch + 1) * CR],
                                  w1v_d[c0:c0 + 128, :])
                nc.sync.dma_start(w1h_sb[:, ch * CR:(ch + 1) * CR],
                                  w1h_d[c0:c0 + 128, :])
                nc.sync.dma_start(ls_sb[:, ch:ch + 1], ls_d[c0:c0 + 128, :])
            nc.sync.dma_start(w2t_sb[:], w2t_d[:, :])
            nc.sync.dma_start(gb_sb[:], gb_d[:, :])

            # persistent small tensors
            s_h_sb = persist.tile([128, NCH * B * H_SH], f32)   # row sums
            ph_sb = persist.tile([CR, B * H_SH], f32)           # Ph + gb
            pv_part_sb = persist.tile([CR, B * W], f32)         # local Pv
            pv_sb = persist.tile([CR, B * W], f32)              # reduced Pv

            pv_in_dr = [dram.tile([CR, W], f32, name=f"pv_in{b}",
                                  tag=f"pvi{b}") for b in range(B)]
            pv_out_dr = [dram.tile([CR, W], f32, name=f"pv_out{b}",
                                   tag=f"pvo{b}") for b in range(B)]

            # Warm up the collective firmware: the first AllReduce of a NEFF
            # pays ~35us of startup; eat it under the phase-A DMA ramp.
            warm_in = dram.tile([CR, 4], f32, name="warm_in", tag="wi")
            warm_out = dram.tile([CR, 4], f32, name="warm_out", tag="wo")
            nc.gpsimd.collective_compute(
                "AllReduce", ALU.add,
                replica_groups=[list(range(N_CORES))],
                ins=[warm_in[:].opt()], outs=[warm_out[:].opt()])

            x_tiles = {}   # tile index -> resident SBUF tile (first NCACHE)
            # anchors to pin C-phase engine work after A-phase engine work
            anchors = {"cast": None, "fold": None}

            psA_cm = tc.tile_pool(name="psA", bufs=2, space="PSUM")
            psA = psA_cm.__enter__()
            psC_cm = tc.tile_pool(name="psC", bufs=2, space="PSUM")
            psC = psC_cm.__enter__()

            def emit_A(b):
                """Pooling pass for batch b, ending in its Pv AllReduce."""
                # [16, 512]: two w-copies (even/odd h rows), folded at the end
                psum_pv = psA.tile([CR, 2 * W], f32, name=f"psum_pv{b}",
                                   tag="pv")
                psum_ph = psA.tile([CR, H_SH], f32, name=f"psum_ph{b}",
                                   tag="ph")
                for ch in range(NCH):
                    c0 = ch * 128
                    for hb in range(NHB):
                        ti = _tile_index(b, ch, hb)
                        col = ch * B * H_SH + b * H_SH + hb * HB
                        if ti < NCACHE:
                            # resident fp32 tile + DVE bf16 cast
                            xt = xcache.tile([128, HB * W], f32,
                                             name=f"xc{ti}", tag=f"slot{ti}")
                            x_tiles[ti] = xt
                            nc.sync.dma_start(
                                xt[:],
                                x_d[b, c0:c0 + 128, hb * HB:(hb + 1) * HB, :])
                            nc.vector.tensor_reduce(
                                out=s_h_sb[:, col:col + HB],
                                in_=xt[:].rearrange("p (h w) -> p h w", w=W),
                                axis=mybir.AxisListType.X, op=ALU.add)
                            xbt = xb_pool.tile([128, HB * W], bf16,
                                               name="xb_t", tag="xb")
                            anchors["cast"] = nc.scalar.copy(xbt[:], xt[:])
                        else:
                            # streamed: SWDGE casting DMA loads bf16 only
                            xbt = xb_pool.tile([128, HB * W], bf16,
                                               name="xb_t", tag="xb")
                            nc.gpsimd.dma_start(
                                xbt[:],
                                x_d[b, c0:c0 + 128, hb * HB:(hb + 1) * HB, :])
                            nc.vector.tensor_reduce(
                                out=s_h_sb[:, col:col + HB],
                                in_=xbt[:].rearrange("p (h w) -> p h w", w=W),
                                axis=mybir.AxisListType.X, op=ALU.add)
                        for k in range(HB // 2):
                            nc.tensor.matmul(
                                psum_pv[:, :],
                                w1v_sb[:, ch * CR:(ch + 1) * CR],
                                xbt[:, 2 * k * W:2 * (k + 1) * W],
                                start=(ch == 0 and hb == 0 and k == 0),
                                stop=(ch == NCH - 1 and hb == NHB - 1
                                      and k == HB // 2 - 1))
                for ch in range(NCH):
                    col = ch * B * H_SH + b * H_SH
                    nc.tensor.matmul(
                        psum_ph[:, :],
                        w1h_sb[:, ch * CR:(ch + 1) * CR],
                        s_h_sb[:, col:col + H_SH],
                        start=(ch == 0), stop=(ch == NCH - 1))
                nc.scalar.activation(ph_sb[:, b * H_SH:(b + 1) * H_SH],
                                     psum_ph[:, :], AF.Identity,
                                     bias=gb_sb[:, 0:1], scale=1.0)
                # per-batch AllReduce of the Pv partial (16 KiB) so the
                # collective overlaps the pooling of later batches
                nc.scalar.copy(pv_part_sb[:, b * W:(b + 1) * W],
                               psum_pv[:, 0:W])
                anchors["fold"] = nc.vector.tensor_add(
                    pv_part_sb[:, b * W:(b + 1) * W],
                    pv_part_sb[:, b * W:(b + 1) * W],
                    psum_pv[:, W:2 * W])
                # store + collective live on the gpsimd stream so the Sync
                # engine's load queue is never blocked behind their waits
                nc.gpsimd.dma_start(pv_in_dr[b][:],
                                    pv_part_sb[:, b * W:(b + 1) * W])
                nc.gpsimd.collective_compute(
                    "AllReduce", ALU.add,
                    replica_groups=[list(range(N_CORES))],
                    ins=[pv_in_dr[b][:].opt()],
                    outs=[pv_out_dr[b][:].opt()])

            def emit_C_prefetch(b):
                """Issue batch b's streamed phase-C reloads on Sync; the
                recycled slots' final readers (y-stores, also on Sync)
                precede them in the stream, so no deadlock."""
                for ch in range(NCH):
                    for hb in range(NHB):
                        ti = _tile_index(b, ch, hb)
                        if ti < NCACHE:
                            continue
                        c0 = ch * 128
                        xt = xcache.tile(
                            [128, HB * W], f32, name=f"xs{ti}",
                            tag=f"slot{(ti - NCACHE) % NCACHE}")
                        x_tiles[ti] = xt
                        nc.sync.dma_start(
                            xt[:],
                            x_d[b, c0:c0 + 128, hb * HB:(hb + 1) * HB, :])

            def emit_C(b):
                """Recalibration pass for batch b."""
                nc.sync.dma_start(pv_sb[:, b * W:(b + 1) * W],
                                  pv_out_dr[b][:])
                HWH = 1024   # half-tile free size
                first_trow = True
                first_ts = True
                for hb in range(NHB):
                    tb = t_pool.tile([CR, HB * W], bf16, name="t_t",
                                     tag="tb")
                    for k in range(HB):
                        col = b * H_SH + hb * HB + k
                        tri = nc.scalar.activation(
                            tb[:, k * W:(k + 1) * W],
                            pv_sb[:, b * W:(b + 1) * W],
                            AF.Relu, bias=ph_sb[:, col:col + 1], scale=1.0)
                        if first_trow and anchors["cast"] is not None:
                            # keep every phase-A ScalarE op ahead of phase C
                            add_dep_helper(tri.ins, anchors["cast"].ins,
                                           reason="phaseC after A casts")
                            first_trow = False
                    for ch in range(NCH):
                        c0 = ch * 128
                        ti = _tile_index(b, ch, hb)
                        xt = x_tiles[ti]   # resident or prefetched
                        vb = v_pool.tile([128, HB * W], f32,
                                         name="v_t", tag="vb")
                        ab = a_pool.tile([128, HB * W], bf16,
                                         name="a_t", tag="ab")
                        for half in range(2):
                            hof = half * HWH
                            ps = psC.tile([128, HWH], f32, name="ps_t",
                                          tag="ps")
                            for j in range(2):
                                nc.tensor.matmul(
                                    ps[:, j * 512:(j + 1) * 512],
                                    w2t_sb[:, c0:c0 + 128],
                                    tb[:, hof + j * 512:hof + (j + 1) * 512],
                                    start=True, stop=True)
                            nc.scalar.activation(ab[:, hof:hof + HWH], ps[:],
                                                 AF.Gelu)
                        # v = ls*A + 1 (per-partition ls), then out = x*v
                        tsi = nc.vector.tensor_scalar(
                            out=vb[:], in0=ab[:],
                            scalar1=ls_sb[:, ch:ch + 1], scalar2=1.0,
                            op0=ALU.mult, op1=ALU.add)
                        if first_ts and anchors["fold"] is not None:
                            # keep every phase-A DVE op ahead of phase C
                            add_dep_helper(tsi.ins, anchors["fold"].ins,
                                           reason="phaseC after A reduces")
                            first_ts = False
                        nc.vector.tensor_mul(xt[:], xt[:], vb[:])
                        nc.sync.dma_start(
                            y_d[b, c0:c0 + 128, hb * HB:(hb + 1) * HB, :],
                            xt[:])

            # All pooling first (the collective firmware needs ~70us to
            # come up, so phase C can't start early anyway), then the
            # recalibration batches, each prefetching the reloads of the
            # batch two steps ahead.
            for b in range(B):
                emit_A(b)
            for b in range(B):
                emit_C(b)
                if b + 2 < B:
                    emit_C_prefetch(b + 2)

            psC_cm.__exit__(None, None, None)
            psA_cm.__exit__(None, None, None)
    nc.compile()
    return nc


def _prepare(x, w1, w2, bn_gamma, bn_beta, bn_mean, bn_var, weight_h,
             weight_v, layer_scale):
    import ml_dtypes
    x = np.asarray(x, dtype=np.float32)
    w1 = np.asarray(w1, dtype=np.float32)
    w2 = np.asarray(w2, dtype=np.float32)
    inv_std = 1.0 / np.sqrt(np.asarray(bn_var, np.float32) + BN_EPS)
    gs = np.asarray(bn_gamma, np.float32) * inv_std
    gb = (np.asarray(bn_beta, np.float32)
          - np.asarray(bn_mean, np.float32) * gs)
    w1s = w1 * gs[:, None]                       # BN scale folded (CR, C)
    wh = float(np.asarray(weight_h).reshape(-1)[0])
    wv = float(np.asarray(weight_v).reshape(-1)[0])
    w1h_t = np.ascontiguousarray(w1s.T * (wh / W)).astype(np.float32)
    w1v_t = np.ascontiguousarray(w1s.T * (wv / H)).astype(ml_dtypes.bfloat16)
    w2t = np.ascontiguousarray(w2.T).astype(ml_dtypes.bfloat16)
    ls = np.ascontiguousarray(
        np.asarray(layer_scale, np.float32).reshape(C, 1))
    gb = np.ascontiguousarray(gb.reshape(CR, 1))
    in_maps = []
    for i in range(N_CORES):
        in_maps.append({
            "x": np.ascontiguousarray(x[:, :, i * H_SH:(i + 1) * H_SH, :]),
            "w1v": w1v_t, "w1h": w1h_t, "w2t": w2t, "gb": gb, "ls": ls,
        })
    return in_maps


def _run(in_maps, **kwargs):
    from concourse.bass_utils import run_bass_kernel_spmd
    nc = _build()
    return run_bass_kernel_spmd(nc, in_maps, core_ids=list(range(N_CORES)),
                                **kwargs)


def kernel(x, w1, w2, bn_gamma, bn_beta, bn_mean, bn_var, weight_h,
           weight_v, layer_scale):
    in_maps = _prepare(x, w1, w2, bn_gamma, bn_beta, bn_mean, bn_var,
                       weight_h, weight_v, layer_scale)
    res = _run(in_maps)
    y = np.empty((B, C, H, W), dtype=np.float32)
    for i in range(N_CORES):
        y[:, :, i * H_SH:(i + 1) * H_SH, :] = res.results[i]["y"]
    return y


# revision 42
# speedup vs baseline: 1.3031x; 1.0221x over previous
"""Trainium2 Bass kernel for DSAM-style strip-pooling attention recalibration.

Math (reference):
    S_h = mean(x, axis=W)                      # (B,C,H,1)
    S_v = mean(x, axis=H)                      # (B,C,1,W)
    F   = wh*S_h + wv*S_v                      # broadcast (B,C,H,W)
    Z   = relu(bn(w1 @ F))                     # (B,CR,H,W)
    A   = gelu(w2 @ Z)                         # (B,C,H,W)
    out = x + ls * (x * A) = x * (1 + ls*A)

Key restructuring: w1 is linear, so w1 @ (wh*S_h + wv*S_v) splits into
    Ph[b,o,h] = (wh/W * gs*w1) @ rowsum_w(x)   (per-h 16-vector)
    Pv[b,o,w] = (wv/H * gs*w1) @ colsum_h(x)   (per-w 16-vector)
with the BN affine folded into the weights (gs) and a bias gb. The full
F tensor is never materialized. Then per (h,w):
    t = relu(Ph[:,h] + Pv[:,w]);  A = gelu(w2 @ t);  out = x*(1+ls*A)

Sharding: H is split across the 8 cores (32 rows each). Row sums (for
Ph) are local; column sums (for Pv) are partial per core and combined
with four tiny (16 x 256) AllReduces, one per batch so each collective
overlaps the pooling of later batches. Pv partials are computed on the
TensorEngine by accumulating w1v^T @ x_bf16[b,:,h,:] over local h,
fusing strip pool + 1x1 conv into the matmul.

The first NCACHE x tiles stay resident in SBUF after the pooling pass so
the recalibration pass re-reads only the tail from HBM; streamed
recalibration tiles recycle the cache slots as they free up.
"""

import functools
import numpy as np

B, C, H, W = 4, 256, 256, 256
CR = 16
N_CORES = 8
H_SH = H // N_CORES          # 32 h-rows per core
HB = 8                       # h-rows per tile
NHB = H_SH // HB             # 4 tile-blocks per core
BN_EPS = 1e-5
NCH = C // 128               # 2 partition chunks of the channel dim
NT = B * NCH * NHB           # 32 x-tiles per core
NCACHE = 14                  # x tiles kept resident between passes


def _tile_index(b, ch, hb):
    return (b * NCH + ch) * NHB + hb


@functools.lru_cache(maxsize=1)
def _build():
    import concourse.bacc as bacc
    import concourse.mybir as mybir
    import concourse.tile as tile

    f32 = mybir.dt.float32
    bf16 = mybir.dt.bfloat16
    AF = mybir.ActivationFunctionType
    ALU = mybir.AluOpType

    from concourse.tile_rust import add_dep_helper

    nc = bacc.Bacc("TRN2", target_bir_lowering=False, debug=False,
                   num_devices=N_CORES)

    x_d = nc.dram_tensor("x", [B, C, H_SH, W], f32, kind="ExternalInput")
    w1v_d = nc.dram_tensor("w1v", [C, CR], bf16, kind="ExternalInput")
    w1h_d = nc.dram_tensor("w1h", [C, CR], f32, kind="ExternalInput")
    w2t_d = nc.dram_tensor("w2t", [CR, C], bf16, kind="ExternalInput")
    gb_d = nc.dram_tensor("gb", [CR, 1], f32, kind="ExternalInput")
    ls_d = nc.dram_tensor("ls", [C, 1], f32, kind="ExternalInput")
    y_d = nc.dram_tensor("y", [B, C, H_SH, W], f32, kind="ExternalOutput")

    with tile.TileContext(nc) as tc:
        with (
            tc.tile_pool(name="consts", bufs=1) as consts,
            tc.tile_pool(name="persist", bufs=1) as persist,
            tc.tile_pool(name="dram", bufs=1, space="DRAM") as dram,
            tc.tile_pool(name="xcache", bufs=1) as xcache,
            tc.tile_pool(name="xb", bufs=3) as xb_pool,
            tc.tile_pool(name="xbs", bufs=6) as xbs_pool,
            tc.tile_pool(name="tb", bufs=3) as t_pool,
            tc.tile_pool(name="ab", bufs=2) as a_pool,
            tc.tile_pool(name="vb", bufs=2) as v_pool,
        ):
            w1v_sb = consts.tile([128, NCH * CR], bf16)
            w1h_sb = consts.tile([128, NCH * CR], f32)
            w2t_sb = consts.tile([CR, C], bf16)
            gb_sb = consts.tile([CR, 1], f32)
            ls_sb = consts.tile([128, NCH], f32)
            for ch in range(NCH):
                c0 = ch * 128
                nc.sync.dma_start(w1v_sb[:, ch * CR:(ch + 1) * CR],
                                  w1v_d[c0:c0 + 128, :])
                nc.sync.dma_start(w1h_sb[:, ch * CR:(ch + 1) * CR],
                                  w1h_d[c0:c0 + 128, :])
                nc.sync.dma_start(ls_sb[:, ch:ch + 1], ls_d[c0:c0 + 128, :])
            nc.sync.dma_start(w2t_sb[:], w2t_d[:, :])
            nc.sync.dma_start(gb_sb[:], gb_d[:, :])

            # persistent small tensors
            s_h_sb = persist.tile([128, NCH * B * H_SH], f32)   # row sums
            ph_sb = persist.tile([CR, B * H_SH], f32)           # Ph + gb
            pv_part_sb = persist.tile([CR, B * W], f32)         # local Pv
            pv_sb = persist.tile([CR, B * W], f32)              # reduced Pv

            pv_in_dr = [dram.tile([CR, W], f32, name=f"pv_in{b}",
                                  tag=f"pvi{b}") for b in range(B)]
            pv_out_dr = [dram.tile([CR, W], f32, name=f"pv_out{b}",
                                   tag=f"pvo{b}") for b in range(B)]

            # Warm up the collective firmware: the first AllReduce of a NEFF
            # pays ~65us of startup; eat it under the phase-A DMA ramp.
            warm_in = dram.tile([CR, 4], f32, name="warm_in", tag="wi")
            warm_out = dram.tile([CR, 4], f32, name="warm_out", tag="wo")
            nc.gpsimd.collective_compute(
                "AllReduce", ALU.add,
                replica_groups=[list(range(N_CORES))],
                ins=[warm_in[:].opt()], outs=[warm_out[:].opt()])

            # All streamed pooling-pass loads (casting SWDGE) issue up
            # front on the gpsimd stream, gated only by xbs slot reuse —
            # never stuck behind a collective trigger.
            xbs_tiles = {}
            for b in range(B):
                for ch in range(NCH):
                    for hb in range(NHB):
                        ti = _tile_index(b, ch, hb)
                        if ti < NCACHE:
                            continue
                        c0 = ch * 128
                        xbt = xbs_pool.tile([128, HB * W], bf16,
                                            name=f"xbs{ti}", tag="xbs")
                        xbs_tiles[ti] = xbt
                        nc.gpsimd.dma_start(
                            xbt[:],
                            x_d[b, c0:c0 + 128, hb * HB:(hb + 1) * HB, :])

            x_tiles = {}   # tile index -> resident SBUF tile (first NCACHE)
            # anchors to pin C-phase engine work after A-phase engine work
            anchors = {"cast": None, "fold": None}

            psA_cm = tc.tile_pool(name="psA", bufs=2, space="PSUM")
            psA = psA_cm.__enter__()
            psC_cm = tc.tile_pool(name="psC", bufs=2, space="PSUM")
            psC = psC_cm.__enter__()

            def emit_A(b):
                """Pooling pass for batch b, ending in its Pv AllReduce."""
                # [16, 512]: two w-copies (even/odd h rows), folded at the end
                psum_pv = psA.tile([CR, 2 * W], f32, name=f"psum_pv{b}",
                                   tag="pv")
                psum_ph = psA.tile([CR, H_SH], f32, name=f"psum_ph{b}",
                                   tag="ph")
                for ch in range(NCH):
                    c0 = ch * 128
                    for hb in range(NHB):
                        ti = _tile_index(b, ch, hb)
                        col = ch * B * H_SH + b * H_SH + hb * HB
                        if ti < NCACHE:
                            # resident fp32 tile + DVE bf16 cast
                            xt = xcache.tile([128, HB * W], f32,
                                             name=f"xc{ti}", tag=f"slot{ti}")
                            x_tiles[ti] = xt
                            nc.sync.dma_start(
                                xt[:],
                                x_d[b, c0:c0 + 128, hb * HB:(hb + 1) * HB, :])
                            nc.vector.tensor_reduce(
                                out=s_h_sb[:, col:col + HB],
                                in_=xt[:].rearrange("p (h w) -> p h w", w=W),
                                axis=mybir.AxisListType.X, op=ALU.add)
                            xbt = xb_pool.tile([128, HB * W], bf16,
                                               name="xb_t", tag="xb")
                            anchors["cast"] = nc.scalar.copy(xbt[:], xt[:])
                        else:
                            # streamed: bf16 tile pre-loaded via SWDGE
                            xbt = xbs_tiles[ti]
                            nc.vector.tensor_reduce(
                                out=s_h_sb[:, col:col + HB],
                                in_=xbt[:].rearrange("p (h w) -> p h w", w=W),
                                axis=mybir.AxisListType.X, op=ALU.add)
                        for k in range(HB // 2):
                            nc.tensor.matmul(
                                psum_pv[:, :],
                                w1v_sb[:, ch * CR:(ch + 1) * CR],
                                xbt[:, 2 * k * W:2 * (k + 1) * W],
                                start=(ch == 0 and hb == 0 and k == 0),
                                stop=(ch == NCH - 1 and hb == NHB - 1
                                      and k == HB // 2 - 1))
                for ch in range(NCH):
                    col = ch * B * H_SH + b * H_SH
                    nc.tensor.matmul(
                        psum_ph[:, :],
                        w1h_sb[:, ch * CR:(ch + 1) * CR],
                        s_h_sb[:, col:col + H_SH],
                        start=(ch == 0), stop=(ch == NCH - 1))
                nc.scalar.activation(ph_sb[:, b * H_SH:(b + 1) * H_SH],
                                     psum_ph[:, :], AF.Identity,
                                     bias=gb_sb[:, 0:1], scale=1.0)
                # per-batch AllReduce of the Pv partial (16 KiB) so the
                # collective overlaps the pooling of later batches
                nc.scalar.copy(pv_part_sb[:, b * W:(b + 1) * W],
                               psum_pv[:, 0:W])
                anchors["fold"] = nc.vector.tensor_add(
                    pv_part_sb[:, b * W:(b + 1) * W],
                    pv_part_sb[:, b * W:(b + 1) * W],
                    psum_pv[:, W:2 * W])
                # store + collective live on the gpsimd stream so the Sync
                # engine's load queue is never blocked behind their waits
                nc.gpsimd.dma_start(pv_in_dr[b][:],
                                    pv_part_sb[:, b * W:(b + 1) * W])
                nc.gpsimd.collective_compute(
                    "AllReduce", ALU.add,
                    replica_groups=[list(range(N_CORES))],
                    ins=[pv_in_dr[b][:].opt()],
                    outs=[pv_out_dr[b][:].opt()])

            def emit_C_prefetch(b):
                """Issue batch b's streamed phase-C reloads on Sync; the
                recycled slots' final readers (y-stores, also on Sync)
                precede them in the stream, so no deadlock."""
                for ch in range(NCH):
                    for hb in range(NHB):
                        ti = _tile_index(b, ch, hb)
                        if ti < NCACHE:
                            continue
                        c0 = ch * 128
                        xt = xcache.tile(
                            [128, HB * W], f32, name=f"xs{ti}",
                            tag=f"slot{(ti - NCACHE) % NCACHE}")
                        x_tiles[ti] = xt
                        nc.sync.dma_start(
                            xt[:],
                            x_d[b, c0:c0 + 128, hb * HB:(hb + 1) * HB, :])

            def emit_C(b):
                """Recalibration pass for batch b."""
                nc.sync.dma_start(pv_sb[:, b * W:(b + 1) * W],
                                  pv_out_dr[b][:])
                HWH = 1024   # half-tile free size
                first_trow = True
                first_ts = True
                for hb in range(NHB):
                    tb = t_pool.tile([CR, HB * W], bf16, name="t_t",
                                     tag="tb")
                    for k in range(HB):
                        col = b * H_SH + hb * HB + k
                        tri = nc.scalar.activation(
                            tb[:, k * W:(k + 1) * W],
                            pv_sb[:, b * W:(b + 1) * W],
                            AF.Relu, bias=ph_sb[:, col:col + 1], scale=1.0)
                        if first_trow and anchors["cast"] is not None:
                            # keep every phase-A ScalarE op ahead of phase C
                            add_dep_helper(tri.ins, anchors["cast"].ins,
                                           reason="phaseC after A casts")
                            first_trow = False
                    for ch in range(NCH):
                        c0 = ch * 128
                        ti = _tile_index(b, ch, hb)
                        xt = x_tiles[ti]   # resident or prefetched
                        vb = v_pool.tile([128, HB * W], f32,
                                         name="v_t", tag="vb")
                        ab = a_pool.tile([128, HB * W], bf16,
                                         name="a_t", tag="ab")
                        for half in range(2):
                            hof = half * HWH
                            ps = psC.tile([128, HWH], f32, name="ps_t",
                                          tag="ps")
                            for j in range(2):
                                nc.tensor.matmul(
                                    ps[:, j * 512:(j + 1) * 512],
                                    w2t_sb[:, c0:c0 + 128],
                                    tb[:, hof + j * 512:hof + (j + 1) * 512],
                                    start=True, stop=True)
                            nc.scalar.activation(ab[:, hof:hof + HWH], ps[:],
                                                 AF.Gelu)
                        # v = ls*A + 1 (per-partition ls), then out = x*v
                        tsi = nc.vector.tensor_scalar(
                            out=vb[:], in0=ab[:],
                            scalar1=ls_sb[:, ch:ch + 1], scalar2=1.0,
                            op0=ALU.mult, op1=ALU.add)
                        if first_ts and anchors["fold"] is not None:
                            # keep every phase-A DVE op ahead of phase C
                            add_dep_helper(tsi.ins, anchors["fold"].ins,
                                           reason="phaseC after A reduces")
                            first_ts = False
                        nc.vector.tensor_mul(xt[:], xt[:], vb[:])
                        nc.sync.dma_start(
                            y_d[b, c0:c0 + 128, hb * HB:(hb + 1) * HB, :],
                            xt[:])

            # All pooling first (the collective firmware needs ~70us to
            # come up, so phase C can't start early anyway), then the
            # recalibration batches, each prefetching the reloads of the
            # batch two steps ahead.
            for b in range(B):
                emit_A(b)
            # pf(b) must follow the stores of the slots' previous
            # occupants in the Sync stream (deadlock-free recycling)
            emit_C(0)
            emit_C_prefetch(1)
            emit_C(1)
            emit_C_prefetch(2)
            emit_C_prefetch(3)
            emit_C(2)
            emit_C(3)

            psC_cm.__exit__(None, None, None)
            psA_cm.__exit__(None, None, None)
    nc.compile()
    return nc


def _prepare(x, w1, w2, bn_gamma, bn_beta, bn_mean, bn_var, weight_h,
             weight_v, layer_scale):
    import ml_dtypes
    x = np.asarray(x, dtype=np.float32)
    w1 = np.asarray(w1, dtype=np.float32)
    w2 = np.asarray(w2, dtype=np.float32)
    inv_std = 1.0 / np.sqrt(np.asarray(bn_var, np.float32) + BN_EPS)
    gs = np.asarray(bn_gamma, np.float32) * inv_std
    gb = (np.asarray(bn_beta, np.float32)
          - np.asarray(bn_mean, np.float32) * gs)
    w1s = w1 * gs[:, None]                       # BN scale folded (CR, C)
    wh = float(np.asarray(weight_h).reshape(-1)[0])
    wv = float(np.asarray(weight_v).reshape(-1)[0])
    w1h_t = np.ascontiguousarray(w1s.T * (wh / W)).astype(np.float32)
    w1v_t = np.ascontiguousarray(w1s.T * (wv / H)).astype(ml_dtypes.bfloat16)
    w2t = np.ascontiguousarray(w2.T).astype(ml_dtypes.bfloat16)
    ls = np.ascontiguousarray(
        np.asarray(layer_scale, np.float32).reshape(C, 1))
    gb = np.ascontiguousarray(gb.reshape(CR, 1))
    in_maps = []
    for i in range(N_CORES):
        in_maps.append({
            "x": np.ascontiguousarray(x[:, :, i * H_SH:(i + 1) * H_SH, :]),
            "w1v": w1v_t, "w1h": w1h_t, "w2t": w2t, "gb": gb, "ls": ls,
        })
    return in_maps


def _run(in_maps, **kwargs):
    from concourse.bass_utils import run_bass_kernel_spmd
    nc = _build()
    return run_bass_kernel_spmd(nc, in_maps, core_ids=list(range(N_CORES)),
                                **kwargs)


def kernel(x, w1, w2, bn_gamma, bn_beta, bn_mean, bn_var, weight_h,
           weight_v, layer_scale):
    in_maps = _prepare(x, w1, w2, bn_gamma, bn_beta, bn_mean, bn_var,
                       weight_h, weight_v, layer_scale)
    res = _run(in_maps)
    y = np.empty((B, C, H, W), dtype=np.float32)
    for i in range(N_CORES):
        y[:, :, i * H_SH:(i + 1) * H_SH, :] = res.results[i]["y"]
    return y
